# revision 1
# baseline (speedup 1.0000x reference)
"""Multi-head attention (B=4, N=2048, C=1024, H=16, D=64) on 8 Trainium2 cores.

Sharding: core = b*2 + hg  (b in 0..3 batches, hg in 0..1 head-groups of 8 heads).
Each core computes, for its (b, hg):
    Q^T, K^T   [512, 2048]  (8 heads x 64 dims on partitions, queries on free)
    V          [2048, 512]  (keys on partitions) + ones column (softmax denom)
    per head pair (2 heads stacked on 128 partitions):
        S^T tiles = K_h^T.T @ Q_h^T  (keys on partitions, queries free)
        expS^T = exp(SCALE * S^T)    (ScalarE, scale folded into activation)
        ctx^T/den = [V_h | 1].T @ expS^T   (ones-augmented PV matmul: row 64 = denom)
        normalize: den -> DRAM -> stride-0 broadcast DMA -> fast reciprocal -> mul
    out_partial = ctx^T.T @ wo_hg^T  [2048, 1024]
Host: out[b] = out_partial[b,hg=0] + out_partial[b,hg=1] + bo.

All matmuls run in float32r (full-speed fp32-reduced mode, ~1e-4 rel err).
S^T matmuls for the two heads of a pair are row-packed onto the 128x128 PE
array (K=64 each at base partitions 0/64) and run concurrently.
"""
import sys

sys.path.insert(0, "/opt/trn_rl_repo")

import numpy as np

import concourse.bass as bass  # noqa: F401
import concourse.tile as tile
from concourse import bacc, mybir
from concourse.bass_utils import run_bass_kernel_spmd

P = 128
B, N, C = 4, 2048, 1024
H = 16
D = 64
HG = 2                 # head groups (tensor-parallel dim)
NH = H // HG           # 8 heads per core
CH = NH * D            # 512 ctx channels per core
KO = C // P            # 8 contraction tiles for projections
NKT = N // P           # 16 key tiles
QC = 512               # query chunk (psum bank)
NQC = N // QC          # 4 query chunks
KTG = 2                # key tiles per exp group
NXQ = 4                # x streamed in quarters
SCALE = D ** -0.5

f32 = mybir.dt.float32
f32r = mybir.dt.float32r

_CACHE = {}


def _build(variant="full"):
    """Build + compile the per-core Bass program (same for all 8 cores).

    variant: "full" | "nop" (overhead probe) | "qkv" (phase 1 only) |
             "attn" (phases 1+2, no projection) — for ablation benchmarking.
    """
    if variant in _CACHE:
        return _CACHE[variant]

    nc = bacc.Bacc("TRN2", target_bir_lowering=False, debug=False)

    xt_d = nc.dram_tensor("xt", [KO, P, N], f32r, kind="ExternalInput").ap()
    wq_d = nc.dram_tensor("wq", [KO, P, CH], f32r, kind="ExternalInput").ap()
    wk_d = nc.dram_tensor("wk", [KO, P, CH], f32r, kind="ExternalInput").ap()
    wv_d = nc.dram_tensor("wv", [KO, P, CH], f32r, kind="ExternalInput").ap()
    wo_d = nc.dram_tensor("wo", [CH // P, P, C], f32r, kind="ExternalInput").ap()
    ones_d = nc.dram_tensor("ones", [P, 1], f32r, kind="ExternalInput").ap()
    out_d = nc.dram_tensor("out", [N, C], f32, kind="ExternalOutput").ap()

    with tile.TileContext(nc) as tc:
        with tc.tile_pool(name="persist", bufs=1) as persist:
            if variant == "nop":
                nop_t = persist.tile([P, QC], f32, tag="nop")
                nc.vector.memset(nop_t[:], 0.0)
                nc.sync.dma_start(out_d[0:P, 0:QC], nop_t[:])
            else:
                _build_body(nc, tc, persist, variant,
                            xt_d, wq_d, wk_d, wv_d, wo_d, ones_d, out_d)

    nc.compile()
    _CACHE[variant] = nc
    return nc


def _build_body(nc, tc, persist, variant, xt_d, wq_d, wk_d, wv_d, wo_d, ones_d, out_d):
    qt = persist.tile([P, CH // P, N], f32r, tag="qt")
    kt = persist.tile([P, CH // P, N], f32r, tag="kt")
    v = persist.tile([P, NKT, NH, D + 1], f32r, tag="v")
    ctxt = persist.tile([P, CH // P, N], f32r, tag="ctxt")
    ones = persist.tile([P, 1], f32r, tag="ones")
    nc.sync.dma_start(ones[:], ones_d[:])

    # ---------------- phase 1: QKV projections ----------------
    # x^T streamed in query-quarters to fit SBUF; weights q/k/v resident.
    NQUARTER = N // NXQ
    with (
        tc.tile_pool(name="px", bufs=1) as px,
        tc.tile_pool(name="pw", bufs=1) as pw,
        tc.tile_pool(name="psum1", bufs=4, space="PSUM") as psum1,
    ):
        wts = {}
        for name, wd in (("v", wv_d), ("q", wq_d), ("k", wk_d)):
            w = pw.tile([P, KO, CH], f32r, tag=f"w{name}")
            for ko in range(KO):
                nc.sync.dma_start(w[:, ko, :], wd[ko])
            wts[name] = w

        for quarter in range(NXQ):
            hsl = slice(quarter * NQUARTER, (quarter + 1) * NQUARTER)
            xt = px.tile([P, KO, NQUARTER], f32r, tag="x")
            for ko in range(KO):
                nc.sync.dma_start(xt[:, ko, :], xt_d[ko, :, hsl])

            # V first (attention needs all of V before any PV work)
            for i in range(NQUARTER // P):
                ikt = quarter * (NQUARTER // P) + i
                ps = psum1.tile([P, CH], f32, tag="ps1")
                for ko in range(KO):
                    nc.tensor.matmul(
                        ps[:], xt[:, ko, i * P:(i + 1) * P], wts["v"][:, ko, :],
                        start=(ko == 0), stop=(ko == KO - 1))
                nc.vector.tensor_copy(
                    v[:, ikt, :, 0:D], ps[:].rearrange("p (h d) -> p h d", d=D))

            # Q, K per m-tile (head pair)
            for mt in range(CH // P):
                for name, dst in (("q", qt), ("k", kt)):
                    for i in range(NQUARTER // QC):
                        qc = quarter * (NQUARTER // QC) + i
                        ps = psum1.tile([P, QC], f32, tag="ps1")
                        for ko in range(KO):
                            nc.tensor.matmul(
                                ps[:],
                                wts[name][:, ko, mt * P:(mt + 1) * P],
                                xt[:, ko, i * QC:(i + 1) * QC],
                                start=(ko == 0), stop=(ko == KO - 1))
                        nc.vector.tensor_copy(dst[:, mt, qc * QC:(qc + 1) * QC], ps[:])
    # ones column (col 64) for all key tiles / heads in one op
    nc.vector.tensor_copy(
        v[:, :, :, D:D + 1],
        ones[:].unsqueeze(1).unsqueeze(1).broadcast_to([P, NKT, NH, 1]))

    if variant == "qkv":
        ot = persist.tile([P, QC], f32, tag="dump")
        nc.vector.tensor_copy(
            ot[:].rearrange("p (h d) -> p h d", d=D), v[:, 0, :, 0:D].bitcast(f32))
        nc.sync.dma_start(out_d[0:P, 0:QC], ot[:])
        return

    # ---------------- phases 2+3: attention with interleaved projection ----
    # qc-outer / head-pair-inner; after each 512-query block's attention
    # completes for all head pairs, its output projection is emitted so the
    # PE-only projection work fills the ScalarE-bound attention slack.
    # Projection PSUM accumulators share the ppv pool slots (same shape/tag).
    with (
        tc.tile_pool(name="pwo", bufs=1) as pwo,
        tc.tile_pool(name="pe", bufs=2) as pe_pool,
        tc.tile_pool(name="pden", bufs=2) as pden,
        tc.tile_pool(name="pout", bufs=4) as pout,
        tc.tile_pool(name="pdram", bufs=2, space="DRAM") as pdram,
    ):
        wo = pwo.tile([P, CH // P, C], f32r, tag="wo")
        for ct in range(CH // P):
            nc.sync.dma_start(wo[:, ct, :], wo_d[ct])

        with (
            tc.tile_pool(name="pst", bufs=1, space="PSUM") as pst,
            tc.tile_pool(name="ppv", bufs=2, space="PSUM") as ppv,
        ):
            for qc in range(NQC):
                qsl = slice(qc * QC, (qc + 1) * QC)
                for hp in range(CH // P):    # head pair = partition tile of qt/kt
                    psA = ppv.tile([P, QC], f32, tag="pvA")
                    psB = ppv.tile([P, QC], f32, tag="pvB")
                    for g in range(NKT // KTG):
                        stA = pst.tile([P, KTG, QC], f32, tag="stA")
                        stB = pst.tile([P, KTG, QC], f32, tag="stB")
                        for j in range(KTG):
                            ik = g * KTG + j
                            ksl = slice(ik * P, (ik + 1) * P)
                            nc.tensor.matmul(stA[:, j, :], kt[0:D, hp, ksl],
                                             qt[0:D, hp, qsl], start=True, stop=True)
                            nc.tensor.matmul(stB[:, j, :], kt[D:P, hp, ksl],
                                             qt[D:P, hp, qsl], start=True, stop=True)
                        eA = pe_pool.tile([P, KTG, QC], f32r, tag="eA")
                        eB = pe_pool.tile([P, KTG, QC], f32r, tag="eB")
                        nc.scalar.activation(eA[:], stA[:],
                                             mybir.ActivationFunctionType.Exp, scale=SCALE)
                        nc.scalar.activation(eB[:], stB[:],
                                             mybir.ActivationFunctionType.Exp, scale=SCALE)
                        for j in range(KTG):
                            ik = g * KTG + j
                            first, last = ik == 0, ik == NKT - 1
                            nc.tensor.matmul(psA[0:D + 1, :], v[:, ik, 2 * hp, :],
                                             eA[:, j, :], start=first, stop=last)
                            nc.tensor.matmul(psB[0:D + 1, :], v[:, ik, 2 * hp + 1, :],
                                             eB[:, j, :], start=first, stop=last)
                    # softmax denominators: stage to DRAM (same-partition copy
                    # first), broadcast back across partitions via stride-0
                    # DMA, batched fast reciprocal, then normalize.
                    den = pden.tile([P, 2, QC], f32, tag="den")
                    nc.vector.tensor_copy(den[D:D + 1, 0, :], psA[D:D + 1, :])
                    nc.vector.tensor_copy(den[D:D + 1, 1, :], psB[D:D + 1, :])
                    den_dr = pdram.tile([2, QC], f32, tag="den_dr")
                    nc.sync.dma_start(den_dr[:], den[D:D + 1, :, :])
                    bcr = pden.tile([P, QC], f32, tag="bcr")
                    nc.sync.dma_start(bcr[0:D, :], den_dr[0].partition_broadcast(D))
                    nc.sync.dma_start(bcr[D:P, :], den_dr[1].partition_broadcast(D))
                    bc = pden.tile([P, QC], f32, tag="bc")
                    nc.vector.reciprocal_approx_fast(bc[:], bcr[:])
                    nc.vector.tensor_mul(ctxt[0:D, hp, qsl], psA[0:D, :], bc[0:D, :])
                    nc.vector.tensor_mul(ctxt[D:P, hp, qsl], psB[0:D, :], bc[D:P, :])

                if variant == "attn":
                    continue
                # projection for this 512-query block (PSUM slots shared with pvA)
                for qt_i in range(4 * qc, 4 * qc + 4):
                    for nt in range(C // QC):
                        ps = ppv.tile([P, QC], f32, tag="pvA")
                        for ct in range(CH // P):
                            nc.tensor.matmul(
                                ps[:], ctxt[:, ct, qt_i * P:(qt_i + 1) * P],
                                wo[:, ct, nt * QC:(nt + 1) * QC],
                                start=(ct == 0), stop=(ct == CH // P - 1))
                        ot = pout.tile([P, QC], f32, tag="ot")
                        nc.vector.tensor_copy(ot[:], ps[:])
                        nc.sync.dma_start(
                            out_d[qt_i * P:(qt_i + 1) * P, nt * QC:(nt + 1) * QC], ot[:])

        if variant == "attn":
            ot = persist.tile([P, QC], f32, tag="dump")
            nc.vector.tensor_copy(ot[:], ctxt[:, 0, 0:QC].bitcast(f32))
            nc.sync.dma_start(out_d[0:P, 0:QC], ot[:])


def _prepare_in_maps(x, wq, wk, wv, wo):
    x = np.ascontiguousarray(np.asarray(x, dtype=np.float32))
    ws = {}
    for hg in range(HG):
        sl = slice(hg * CH, (hg + 1) * CH)
        ws[hg] = {
            "wq": np.ascontiguousarray(np.asarray(wq)[sl, :].T).reshape(KO, P, CH),
            "wk": np.ascontiguousarray(np.asarray(wk)[sl, :].T).reshape(KO, P, CH),
            "wv": np.ascontiguousarray(np.asarray(wv)[sl, :].T).reshape(KO, P, CH),
            "wo": np.ascontiguousarray(np.asarray(wo)[:, sl].T).reshape(CH // P, P, C),
        }
    ones = np.ones((P, 1), dtype=np.float32)
    in_maps = []
    for core in range(8):
        b, hg = core // HG, core % HG
        xt = np.ascontiguousarray(x[b].T).reshape(KO, P, N)
        m = {"xt": xt, "ones": ones}
        m.update(ws[hg])
        in_maps.append(m)
    return in_maps


def kernel(x, wq, wk, wv, wo, bo):
    nc = _build()
    in_maps = _prepare_in_maps(x, wq, wk, wv, wo)
    res = run_bass_kernel_spmd(nc, in_maps, core_ids=list(range(8)))
    bo = np.asarray(bo, dtype=np.float32)
    out = np.empty((B, N, C), dtype=np.float32)
    for b in range(B):
        out[b] = res.results[2 * b]["out"] + res.results[2 * b + 1]["out"] + bo
    return out



# revision 10
# speedup vs baseline: 3.6918x; 3.6918x over previous
"""Multi-head attention (B=4, N=2048, C=1024, H=16, D=64) on 8 Trainium2 cores.

Sharding: core = b*2 + hg  (b in 0..3 batches, hg in 0..1 head-groups of 8 heads).
Each core computes, for its (b, hg):
    Q^T, K^T   [512, 2048]  (8 heads x 64 dims on partitions, queries on free)
    V          [2048, 512]  (keys on partitions) + ones column (softmax denom)
    per head pair (2 heads stacked on 128 partitions):
        S^T tiles = K_h^T.T @ Q_h^T  (keys on partitions, queries free)
        expS^T = exp(SCALE * S^T)    (ScalarE, scale folded into activation)
        ctx^T/den = [V_h | 1].T @ expS^T   (ones-augmented PV matmul: row 64 = denom)
        normalize: den -> DRAM -> stride-0 broadcast DMA -> fast reciprocal -> mul
    out_partial = ctx^T.T @ wo_hg^T  [2048, 1024]
Host: out[b] = out_partial[b,hg=0] + out_partial[b,hg=1] + bo.

All matmuls run in float32r (full-speed fp32-reduced mode, ~1e-4 rel err).
S^T matmuls for the two heads of a pair are row-packed onto the 128x128 PE
array (K=64 each at base partitions 0/64) and run concurrently.
"""
import sys

sys.path.insert(0, "/opt/trn_rl_repo")

import numpy as np

import concourse.bass as bass  # noqa: F401
import concourse.tile as tile
from concourse import bacc, mybir
from concourse.bass_utils import run_bass_kernel_spmd

P = 128
B, N, C = 4, 2048, 1024
H = 16
D = 64
HG = 2                 # head groups (tensor-parallel dim)
NH = H // HG           # 8 heads per core
CH = NH * D            # 512 ctx channels per core
KO = C // P            # 8 contraction tiles for projections
NKT = N // P           # 16 key tiles
QC = 512               # query chunk (psum bank)
NQC = N // QC          # 4 query chunks
KTG = 2                # key tiles per exp group
NXQ = 4                # x streamed in quarters
SCALE = D ** -0.5

f32 = mybir.dt.float32
f32r = mybir.dt.float32r

_CACHE = {}


def _build(variant="full"):
    """Build + compile the per-core Bass program (same for all 8 cores).

    variant: "full" | "nop" (overhead probe) | "qkv" (phase 1 only) |
             "attn" (phases 1+2, no projection) — for ablation benchmarking.
    """
    if variant in _CACHE:
        return _CACHE[variant]

    nc = bacc.Bacc("TRN2", target_bir_lowering=False, debug=False)

    xt_d = nc.dram_tensor("xt", [KO, P, N], f32r, kind="ExternalInput").ap()
    wq_d = nc.dram_tensor("wq", [KO, P, CH], f32r, kind="ExternalInput").ap()
    wk_d = nc.dram_tensor("wk", [KO, P, CH], f32r, kind="ExternalInput").ap()
    wv_d = nc.dram_tensor("wv", [KO, P, CH], f32r, kind="ExternalInput").ap()
    wo_d = nc.dram_tensor("wo", [CH // P, P, C], f32r, kind="ExternalInput").ap()
    ones_d = nc.dram_tensor("ones", [P, 1], f32r, kind="ExternalInput").ap()
    out_d = nc.dram_tensor("out", [N, C], f32, kind="ExternalOutput").ap()

    with tile.TileContext(nc) as tc:
        with tc.tile_pool(name="persist", bufs=1) as persist:
            if variant == "nop":
                nop_t = persist.tile([P, QC], f32, tag="nop")
                nc.vector.memset(nop_t[:], 0.0)
                nc.sync.dma_start(out_d[0:P, 0:QC], nop_t[:])
            else:
                _build_body(nc, tc, persist, variant,
                            xt_d, wq_d, wk_d, wv_d, wo_d, ones_d, out_d)

    nc.compile()
    _CACHE[variant] = nc
    return nc


def _build_body(nc, tc, persist, variant, xt_d, wq_d, wk_d, wv_d, wo_d, ones_d, out_d):
    i16 = mybir.dt.int16
    bf16 = mybir.dt.bfloat16
    # Schraudolph fast-exp constants (DVE path, bf16 flavor): exp(SCALE*s) ~=
    # bitcast_bf16(i16(s * (2^7/ln2 * SCALE) + (127*2^7 - 366393/2^16)))
    EXP_A = float(2 ** 7 / np.log(2) * SCALE)
    EXP_B = float(127 * 2 ** 7 - 366393 / 2 ** 16)

    qt = persist.tile([P, CH // P, N], f32r, tag="qt")
    kt = persist.tile([P, CH // P, N], f32r, tag="kt")
    v = persist.tile([P, NKT, NH, D + 1], bf16, tag="v")
    ctxt = persist.tile([P, CH // P, N], f32r, tag="ctxt")
    ones = persist.tile([P, 1], f32r, tag="ones")
    nc.sync.dma_start(ones[:], ones_d[:])

    # ---------------- phase 1: QKV projections ----------------
    # x^T streamed in query-quarters to fit SBUF; weights q/k/v resident.
    NQUARTER = N // NXQ
    with (
        tc.tile_pool(name="px", bufs=1) as px,
        tc.tile_pool(name="pw", bufs=1) as pw,
        tc.tile_pool(name="psum1", bufs=4, space="PSUM") as psum1,
    ):
        wts = {}
        for name, wd in (("v", wv_d), ("q", wq_d), ("k", wk_d)):
            w = pw.tile([P, KO, CH], f32r, tag=f"w{name}")
            for ko in range(KO):
                nc.sync.dma_start(w[:, ko, :], wd[ko])
            wts[name] = w

        for quarter in range(NXQ):
            hsl = slice(quarter * NQUARTER, (quarter + 1) * NQUARTER)
            xt = px.tile([P, KO, NQUARTER], f32r, tag="x")
            for ko in range(KO):
                nc.sync.dma_start(xt[:, ko, :], xt_d[ko, :, hsl])

            # V first (attention needs all of V before any PV work)
            for i in range(NQUARTER // P):
                ikt = quarter * (NQUARTER // P) + i
                ps = psum1.tile([P, CH], f32, tag="ps1")
                for ko in range(KO):
                    nc.tensor.matmul(
                        ps[:], xt[:, ko, i * P:(i + 1) * P], wts["v"][:, ko, :],
                        start=(ko == 0), stop=(ko == KO - 1))
                nc.vector.tensor_copy(
                    v[:, ikt, :, 0:D], ps[:].rearrange("p (h d) -> p h d", d=D))

            # Q, K per m-tile (head pair)
            for mt in range(CH // P):
                for name, dst in (("q", qt), ("k", kt)):
                    for i in range(NQUARTER // QC):
                        qc = quarter * (NQUARTER // QC) + i
                        ps = psum1.tile([P, QC], f32, tag="ps1")
                        for ko in range(KO):
                            nc.tensor.matmul(
                                ps[:],
                                wts[name][:, ko, mt * P:(mt + 1) * P],
                                xt[:, ko, i * QC:(i + 1) * QC],
                                start=(ko == 0), stop=(ko == KO - 1))
                        nc.vector.tensor_copy(dst[:, mt, qc * QC:(qc + 1) * QC], ps[:])
    # ones column (col 64) for all key tiles / heads in one op
    nc.vector.tensor_copy(
        v[:, :, :, D:D + 1],
        ones[:].unsqueeze(1).unsqueeze(1).broadcast_to([P, NKT, NH, 1]))

    if variant == "qkv":
        ot = persist.tile([P, QC], f32, tag="dump")
        nc.vector.tensor_copy(
            ot[:].rearrange("p (h d) -> p h d", d=D), v[:, 0, :, 0:D])
        nc.sync.dma_start(out_d[0:P, 0:QC], ot[:])
        return

    # ---------------- phases 2+3: attention with interleaved projection ----
    # qc-outer / head-pair-inner; after each 512-query block's attention
    # completes for all head pairs, its output projection is emitted so the
    # PE-only projection work fills the ScalarE-bound attention slack.
    # Projection PSUM accumulators share the ppv pool slots (same shape/tag).
    with (
        tc.tile_pool(name="pwo", bufs=1) as pwo,
        tc.tile_pool(name="pe", bufs=2) as pe_pool,
        tc.tile_pool(name="pden", bufs=2) as pden,
        tc.tile_pool(name="pout", bufs=4) as pout,
        tc.tile_pool(name="pdram", bufs=2, space="DRAM") as pdram,
    ):
        wo = pwo.tile([P, CH // P, C], f32r, tag="wo")
        for ct in range(CH // P):
            nc.sync.dma_start(wo[:, ct, :], wo_d[ct])

        with (
            tc.tile_pool(name="pst", bufs=2, space="PSUM") as pst,
            tc.tile_pool(name="ppv", bufs=2, space="PSUM") as ppv,
        ):
            ecount = 0  # running exp-instruction counter for ACT/DVE split

            def emit_exp(e_tile, st_tile):
                """2 of every 7 exp instructions run on DVE (Schraudolph
                bit-trick, one fused mult+add with i32-convert output); the
                rest on ACT. e_tile is i32; consumers bitcast to f32r."""
                nonlocal ecount
                if ecount % 7 in (2, 5):
                    nc.vector.tensor_scalar(
                        out=e_tile[:], in0=st_tile[:], scalar1=EXP_A,
                        scalar2=EXP_B, op0=mybir.AluOpType.mult,
                        op1=mybir.AluOpType.add)
                else:
                    nc.scalar.activation(e_tile[:].bitcast(bf16), st_tile[:],
                                         mybir.ActivationFunctionType.Exp,
                                         scale=SCALE)
                ecount += 1

            for qc in range(NQC):
                qsl = slice(qc * QC, (qc + 1) * QC)
                for hp in range(CH // P):    # head pair = partition tile of qt/kt
                    psA = ppv.tile([P, QC], f32, tag="pvA")
                    psB = ppv.tile([P, QC], f32, tag="pvB")
                    for ik in range(NKT):
                        ksl = slice(ik * P, (ik + 1) * P)
                        stA = pst.tile([P, QC], f32, tag="stA")
                        stB = pst.tile([P, QC], f32, tag="stB")
                        nc.tensor.matmul(stA[:], kt[0:D, hp, ksl],
                                         qt[0:D, hp, qsl], start=True, stop=True)
                        nc.tensor.matmul(stB[:], kt[D:P, hp, ksl],
                                         qt[D:P, hp, qsl], start=True, stop=True)
                        eA = pe_pool.tile([P, QC], i16, tag="eA")
                        eB = pe_pool.tile([P, QC], i16, tag="eB")
                        emit_exp(eA, stA)
                        emit_exp(eB, stB)
                        first, last = ik == 0, ik == NKT - 1
                        nc.tensor.matmul(psA[0:D + 1, :], v[:, ik, 2 * hp, :],
                                         eA[:].bitcast(bf16),
                                         start=first, stop=last)
                        nc.tensor.matmul(psB[0:D + 1, :], v[:, ik, 2 * hp + 1, :],
                                         eB[:].bitcast(bf16),
                                         start=first, stop=last)
                    # softmax denominators: stage to DRAM (same-partition copy
                    # first), broadcast back across partitions via stride-0
                    # DMA, batched fast reciprocal, then normalize.
                    den = pden.tile([P, 2, QC], f32, tag="den")
                    nc.vector.tensor_copy(den[D:D + 1, 0, :], psA[D:D + 1, :])
                    nc.vector.tensor_copy(den[D:D + 1, 1, :], psB[D:D + 1, :])
                    den_dr = pdram.tile([2, QC], f32, tag="den_dr")
                    nc.sync.dma_start(den_dr[:], den[D:D + 1, :, :])
                    bcr = pden.tile([P, QC], f32, tag="bcr")
                    nc.sync.dma_start(bcr[0:D, :], den_dr[0].partition_broadcast(D))
                    nc.sync.dma_start(bcr[D:P, :], den_dr[1].partition_broadcast(D))
                    bc = pden.tile([P, QC], f32, tag="bc")
                    nc.vector.reciprocal_approx_fast(bc[:], bcr[:])
                    nc.vector.tensor_mul(ctxt[0:D, hp, qsl], psA[0:D, :], bc[0:D, :])
                    nc.vector.tensor_mul(ctxt[D:P, hp, qsl], psB[0:D, :], bc[D:P, :])

                if variant == "attn":
                    continue
                # projection for this 512-query block (PSUM slots shared with pvA)
                for qt_i in range(4 * qc, 4 * qc + 4):
                    for nt in range(C // QC):
                        ps = ppv.tile([P, QC], f32, tag="pvA")
                        for ct in range(CH // P):
                            nc.tensor.matmul(
                                ps[:], ctxt[:, ct, qt_i * P:(qt_i + 1) * P],
                                wo[:, ct, nt * QC:(nt + 1) * QC],
                                start=(ct == 0), stop=(ct == CH // P - 1))
                        ot = pout.tile([P, QC], f32, tag="ot")
                        nc.vector.tensor_copy(ot[:], ps[:])
                        nc.sync.dma_start(
                            out_d[qt_i * P:(qt_i + 1) * P, nt * QC:(nt + 1) * QC], ot[:])

        if variant == "attn":
            ot = persist.tile([P, QC], f32, tag="dump")
            nc.vector.tensor_copy(ot[:], ctxt[:, 0, 0:QC].bitcast(f32))
            nc.sync.dma_start(out_d[0:P, 0:QC], ot[:])


def _prepare_in_maps(x, wq, wk, wv, wo):
    x = np.ascontiguousarray(np.asarray(x, dtype=np.float32))
    ws = {}
    for hg in range(HG):
        sl = slice(hg * CH, (hg + 1) * CH)
        ws[hg] = {
            "wq": np.ascontiguousarray(np.asarray(wq)[sl, :].T).reshape(KO, P, CH),
            "wk": np.ascontiguousarray(np.asarray(wk)[sl, :].T).reshape(KO, P, CH),
            "wv": np.ascontiguousarray(np.asarray(wv)[sl, :].T).reshape(KO, P, CH),
            "wo": np.ascontiguousarray(np.asarray(wo)[:, sl].T).reshape(CH // P, P, C),
        }
    ones = np.ones((P, 1), dtype=np.float32)
    in_maps = []
    for core in range(8):
        b, hg = core // HG, core % HG
        xt = np.ascontiguousarray(x[b].T).reshape(KO, P, N)
        m = {"xt": xt, "ones": ones}
        m.update(ws[hg])
        in_maps.append(m)
    return in_maps


def kernel(x, wq, wk, wv, wo, bo):
    nc = _build()
    in_maps = _prepare_in_maps(x, wq, wk, wv, wo)
    res = run_bass_kernel_spmd(nc, in_maps, core_ids=list(range(8)))
    bo = np.asarray(bo, dtype=np.float32)
    out = np.empty((B, N, C), dtype=np.float32)
    for b in range(B):
        out[b] = res.results[2 * b]["out"] + res.results[2 * b + 1]["out"] + bo
    return out

